# revision 26
# baseline (speedup 1.0000x reference)
# MinGRU block kernel for 8 Trainium2 NeuronCores (Bass/Tile).
#
# Reference computation (B=4, L=8192, D=1024, f32):
#   norm = rmsnorm(inp, ln_w)
#   beta = sigmoid(norm @ Wg.T); hx_hat = norm @ Wc.T
#   a = 1-beta; x = beta*hx_hat
#   h = assoc_scan(h_t = a_t*h_{t-1} + x_t) along L
#   out = h + SwiGLU_FFN(rmsnorm(h, ffn_w));  returns (out, h)
#
# Sharding: 8 cores = 4 batches x 2 sequence halves. Scan carry between the
# halves is exchanged on host between two launches.
#   L1: xn = inp^T * inv_rms (host-precomputed scale) -> gate matmuls (bf16)
#       -> a = sigmoid(-g) on scalar engine, x = beta*c on DVE -> local scan
#       (for the carry) -> packed a,x bf16 to DRAM.
#   L2: scan(a, x, init=carry) -> rmsnorm (PE ones-matmul + gpsimd bcast)
#       -> SwiGLU FFN in fp8e4 DoubleRow perf mode (2x PE) -> out^T bf16.
# Both launches software-pipeline tile i+1's load/scan/norm phases under
# tile i's matmul phases so the tensor engine never waits on the scan chain.
# ln_w / ffn_w / fp8 scale (x32) are folded into the weights on host.

import sys

sys.path.insert(0, "/opt/trn_rl_repo")

import numpy as np
import ml_dtypes

import concourse.bass as bass
import concourse.tile as tile
from concourse import mybir, bacc
from concourse.bass_utils import run_bass_kernel_spmd

B, L, D = 4, 8192, 1024
NCORES = 8
T = L // 2        # tokens per core
TT = 512          # token tile
NT = T // TT      # 8 token tiles per core
KC = D // 128     # contraction chunks
EC = D // 128     # output-channel chunks
EPS = 1e-6
WS = 32.0         # fp8 weight scale (power of 2)

f32 = mybir.dt.float32
bf16 = mybir.dt.bfloat16
fp8 = mybir.dt.float8e4
AF = mybir.ActivationFunctionType
OP = mybir.AluOpType
DR = mybir.MatmulPerfMode.DoubleRow
bf16_np = ml_dtypes.bfloat16
fp8_np = ml_dtypes.float8_e4m3


def build_l1():
    nc = bacc.Bacc(None, target_bir_lowering=False)
    xT_d = nc.dram_tensor("xT", [D, T], f32, kind="ExternalInput")
    inv_d = nc.dram_tensor("inv", [128, T], f32, kind="ExternalInput")
    # weights pre-packed on host as [EC, 128, KC, 128] (e-chunk major) so the
    # e=0 chains only wait for 0.5 MiB of weight DMA at launch
    wgP_d = nc.dram_tensor("wgP", [EC, 128, KC, 128], bf16, kind="ExternalInput")
    wcP_d = nc.dram_tensor("wcP", [EC, 128, KC, 128], bf16, kind="ExternalInput")
    ax_T = nc.dram_tensor("ax_T", [D, 2, T], bf16, kind="ExternalOutput")
    hlast = nc.dram_tensor("hlast", [128, EC], f32, kind="ExternalOutput")

    xT_r = xT_d[:].rearrange("(k p) t -> p k t", p=128)

    with tile.TileContext(nc) as tc:
        with (
            tc.tile_pool(name="wpool", bufs=1) as wpool,
            tc.tile_pool(name="xin", bufs=2) as xin,
            tc.tile_pool(name="invp", bufs=2) as invp,
            tc.tile_pool(name="xnp", bufs=2) as xnp,
            tc.tile_pool(name="gate", bufs=4) as gate,
            tc.tile_pool(name="scan", bufs=2) as scanp,
            tc.tile_pool(name="per", bufs=1) as per,
            tc.tile_pool(name="psg", bufs=2, space=bass.MemorySpace.PSUM) as psg,
            tc.tile_pool(name="psc", bufs=2, space=bass.MemorySpace.PSUM) as psc,
        ):
            hprev = per.tile([128, EC], f32)
            nc.vector.memset(hprev[:], 0.0)

            def load_tile(i):
                # per-k DMA chunks so the first chains can start before the
                # whole tile has landed
                ts = slice(i * TT, (i + 1) * TT)
                invt = invp.tile([128, TT], f32)
                nc.sync.dma_start(invt[:], inv_d[:, ts])
                xt = xin.tile([128, KC, TT], f32)
                for k in range(KC):
                    nc.sync.dma_start(xt[:, k, :], xT_r[:, k, ts])
                return xt, invt

            def make_xn(xt, invt, eng):
                # steady state runs on gpsimd: not latency-critical (issued a
                # tile ahead) and keeps the DVE free for the scan chain
                xn = xnp.tile([128, KC, TT], bf16)
                for k in range(KC):
                    eng.tensor_mul(xn[:, k, :], xt[:, k, :], invt[:])
                return xn

            # DMA order: inv+first x chunk, then the e=0 weights (so the first
            # chain starts ~2us in), then the rest of the inputs and weights
            inv0 = invp.tile([128, TT], f32)
            nc.sync.dma_start(inv0[:], inv_d[:, 0:TT])
            xt0 = xin.tile([128, KC, TT], f32)
            nc.sync.dma_start(xt0[:, 0, :], xT_r[:, 0, 0:TT])
            wg_sb = wpool.tile([128, EC, KC, 128], bf16)
            wc_sb = wpool.tile([128, EC, KC, 128], bf16)
            nc.sync.dma_start(wg_sb[:, 0], wgP_d[:][0])
            nc.sync.dma_start(wc_sb[:, 0], wcP_d[:][0])
            for k in range(1, KC):
                nc.sync.dma_start(xt0[:, k, :], xT_r[:, k, 0:TT])
            for e in range(1, EC):
                nc.sync.dma_start(wg_sb[:, e], wgP_d[:][e])
                nc.sync.dma_start(wc_sb[:, e], wcP_d[:][e])
            nxt = (xt0, inv0)
            xn = make_xn(*nxt, nc.vector)

            for i in range(NT):
                ts = slice(i * TT, (i + 1) * TT)
                if i + 1 < NT:
                    nxt = load_tile(i + 1)
                xn_cur, xn = xn, None
                for e in range(EC):
                    es = slice(e * 128, (e + 1) * 128)
                    pm_g = psg.tile([128, TT], f32)
                    for k in range(KC):
                        nc.tensor.matmul(
                            pm_g[:], wg_sb[:, e, k, :], xn_cur[:, k, :],
                            start=(k == 0), stop=(k == KC - 1),
                        )
                    pm_c = psc.tile([128, TT], f32)
                    for k in range(KC):
                        nc.tensor.matmul(
                            pm_c[:], wc_sb[:, e, k, :], xn_cur[:, k, :],
                            start=(k == 0), stop=(k == KC - 1),
                        )
                    ax = gate.tile([128, 2, TT], bf16, tag="ax")
                    nc.scalar.activation(ax[:, 0, :], pm_g[:], AF.Sigmoid, scale=-1.0)
                    beta = gate.tile([128, TT], bf16, tag="beta")
                    nc.scalar.activation(beta[:], pm_g[:], AF.Sigmoid)
                    nc.vector.tensor_mul(ax[:, 1, :], beta[:], pm_c[:])
                    nc.sync.dma_start(ax_T[es, :, ts], ax[:])

                    h = scanp.tile([128, TT], f32)
                    nc.vector.tensor_tensor_scan(
                        h[:], ax[:, 0, :], ax[:, 1, :], hprev[:, e : e + 1],
                        OP.mult, OP.add,
                    )
                    nc.vector.tensor_copy(hprev[:, e : e + 1], h[:, TT - 1 : TT])
                    if e == 0 and i + 1 < NT:
                        xn = make_xn(*nxt, nc.gpsimd)

            nc.sync.dma_start(hlast[:], hprev[:])

    nc.compile()
    return nc


def build_l2():
    nc = bacc.Bacc(None, target_bir_lowering=False)
    ax_in = nc.dram_tensor("ax_in", [D, 2, T], bf16, kind="ExternalInput")
    carry_d = nc.dram_tensor("carry", [128, EC], f32, kind="ExternalInput")
    w1T_d = nc.dram_tensor("w1T", [D, D], fp8, kind="ExternalInput")
    w3T_d = nc.dram_tensor("w3T", [D, D], fp8, kind="ExternalInput")
    w2T_d = nc.dram_tensor("w2T", [D, D], fp8, kind="ExternalInput")
    out_sT = nc.dram_tensor("out_sT", [D, T], bf16, kind="ExternalOutput")
    hx_sT = nc.dram_tensor("hx_sT", [D, T], bf16, kind="ExternalOutput")

    with tile.TileContext(nc) as tc:
        with (
            tc.tile_pool(name="wpool", bufs=1) as wpool,
            tc.tile_pool(name="ax", bufs=4) as axp,
            tc.tile_pool(name="hp", bufs=3) as hp,
            tc.tile_pool(name="hxb", bufs=4) as hxbp,
            tc.tile_pool(name="nrm", bufs=2) as nrm,
            tc.tile_pool(name="hnp", bufs=2) as hnp,
            tc.tile_pool(name="ffn", bufs=2) as ffnp,
            tc.tile_pool(name="outp", bufs=4) as outp,
            tc.tile_pool(name="per", bufs=1) as per,
            tc.tile_pool(name="ps1", bufs=2, space=bass.MemorySpace.PSUM) as ps1,
            tc.tile_pool(name="ps3", bufs=2, space=bass.MemorySpace.PSUM) as ps3,
            tc.tile_pool(name="psf", bufs=2, space=bass.MemorySpace.PSUM) as psf,
            tc.tile_pool(name="pss", bufs=1, space=bass.MemorySpace.PSUM) as pss,
            tc.tile_pool(name="psr", bufs=1, space=bass.MemorySpace.PSUM) as psr,
        ):
            hprev = per.tile([128, EC], f32)
            carry_sb = per.tile([128, EC], f32)
            nc.sync.dma_start(carry_sb[:], carry_d[:])
            eps_row = per.tile([1, 1], f32)
            nc.vector.memset(eps_row[:], EPS)
            ones_b = per.tile([128, 1], bf16)
            nc.vector.memset(ones_b[:], 1.0)
            ones_c = per.tile([1, 128], f32)
            nc.vector.memset(ones_c[:], 1.0)

            def scan_chunk(i, c, h_t):
                # DMA load + DVE scan + hx cast/write (gpsimd) + square
                # (scalar) for one channel chunk of tile i
                ts = slice(i * TT, (i + 1) * TT)
                cs = slice(c * 128, (c + 1) * 128)
                ax = axp.tile([128, 2, TT], bf16)
                nc.sync.dma_start(ax[:], ax_in[cs, :, ts])
                init = carry_sb[:, c : c + 1] if i == 0 else hprev[:, c : c + 1]
                nc.vector.tensor_tensor_scan(
                    h_t[:, c, :], ax[:, 0, :], ax[:, 1, :], init, OP.mult, OP.add
                )
                nc.vector.tensor_copy(hprev[:, c : c + 1], h_t[:, c, TT - 1 : TT])
                hxb = hxbp.tile([128, TT], bf16)
                nc.gpsimd.tensor_copy(hxb[:], h_t[:, c, :])
                nc.sync.dma_start(hx_sT[cs, ts], hxb[:])
                hsq = nrm.tile([128, TT], bf16, tag=f"hsq{c}")
                nc.scalar.square(hsq[:], h_t[:, c, :])
                return hsq

            def ssq_sqrt(hsqs):
                # PE ones-matmul ssq reduction + scalar sqrt -> rms row
                pm_ssq = pss.tile([1, TT], f32)
                for c in range(EC):
                    nc.tensor.matmul(
                        pm_ssq[:], ones_b[:], hsqs[c][:],
                        start=(c == 0), stop=(c == EC - 1),
                    )
                rms_row = nrm.tile([1, TT], f32, tag="rmsrow")
                nc.scalar.activation(
                    rms_row[:], pm_ssq[:], AF.Sqrt, scale=1.0 / D, bias=eps_row[:]
                )
                return rms_row

            def norm_b(h_t, rms_row):
                # PE broadcast -> 1/rms (fast approx) -> hn in fp8 (DVE)
                pm_rms = psr.tile([128, TT], f32)
                nc.tensor.matmul(pm_rms[:], ones_c[:], rms_row[:], start=True, stop=True)
                inv_bc = nrm.tile([128, TT], f32, tag="invbc")
                nc.vector.reciprocal_approx_fast(inv_bc[:], pm_rms[:])
                hn_t = hnp.tile([128, KC, TT], fp8)
                for c in range(EC):
                    nc.vector.tensor_mul(hn_t[:, c, :], h_t[:, c, :], inv_bc[:])
                return hn_t

            # prologue: tile 0 scan + full norm, tile 1 scan; weights DMA
            # after tile 0's a,x loads (they are not needed until the FFN)
            h_map, hsq_map = {}, {}
            h_map[0] = hp.tile([128, EC, TT], f32, name="h_t", tag="h_t")
            hsq_map[0] = [scan_chunk(0, c, h_map[0]) for c in range(EC)]
            w1_sb = wpool.tile([128, KC, D], fp8)
            nc.sync.dma_start(w1_sb[:], w1T_d[:].rearrange("(k p) e -> p k e", p=128))
            w3_sb = wpool.tile([128, KC, D], fp8)
            nc.sync.dma_start(w3_sb[:], w3T_d[:].rearrange("(k p) e -> p k e", p=128))
            w2_sb = wpool.tile([128, KC, D], fp8)
            nc.sync.dma_start(w2_sb[:], w2T_d[:].rearrange("(k p) e -> p k e", p=128))
            hn_t = norm_b(h_map[0], ssq_sqrt(hsq_map[0]))
            if NT > 1:
                h_map[1] = hp.tile([128, EC, TT], f32, name="h_t", tag="h_t")
                hsq_map[1] = [scan_chunk(1, c, h_map[1]) for c in range(EC)]

            for i in range(NT):
                ts = slice(i * TT, (i + 1) * TT)
                h_cur, hn_cur = h_map.pop(i), hn_t

                if i + 2 < NT:
                    h_map[i + 2] = hp.tile([128, EC, TT], f32, name="h_t", tag="h_t")
                    hsq_map[i + 2] = []

                # FFN stage 1: u = silu(w1@hn/32) * (w3@hn/32), fp8 DoubleRow.
                # Tile i+2's scan chunks interleave (two-tile skew) so the
                # scan chain never races the FFN that consumes it.
                u_t = ffnp.tile([128, KC, TT], fp8, tag="u")
                for e in range(EC):
                    es = slice(e * 128, (e + 1) * 128)
                    pm_1 = ps1.tile([128, TT], f32)
                    for kk in range(KC // 2):
                        nc.tensor.matmul(
                            pm_1[:],
                            w1_sb[:, 2 * kk : 2 * kk + 2, es],
                            hn_cur[:, 2 * kk : 2 * kk + 2, :],
                            start=(kk == 0), stop=(kk == KC // 2 - 1),
                            perf_mode=DR,
                        )
                    sil = ffnp.tile([128, TT], bf16, tag="sil")
                    nc.scalar.activation(sil[:], pm_1[:], AF.Silu, scale=1.0 / WS)
                    pm_3 = ps3.tile([128, TT], f32)
                    for kk in range(KC // 2):
                        nc.tensor.matmul(
                            pm_3[:],
                            w3_sb[:, 2 * kk : 2 * kk + 2, es],
                            hn_cur[:, 2 * kk : 2 * kk + 2, :],
                            start=(kk == 0), stop=(kk == KC // 2 - 1),
                            perf_mode=DR,
                        )
                    nc.vector.scalar_tensor_tensor(
                        u_t[:, e, :], pm_3[:], 1.0 / WS, sil[:], OP.mult, OP.mult
                    )
                    if i + 2 < NT:
                        hsq_map[i + 2].append(scan_chunk(i + 2, e, h_map[i + 2]))

                # rms row for tile i+1 (PE + scalar; inputs were ready a full
                # tile ago)
                if i + 1 < NT:
                    rms_next = ssq_sqrt(hsq_map.pop(i + 1))

                # FFN stage 2: ff = w2@u/32 + h, store out^T (bf16); tile
                # i+1's broadcast/recip/hn run under the remaining w2 chains
                for e in range(EC):
                    es = slice(e * 128, (e + 1) * 128)
                    pm_f = psf.tile([128, TT], f32)
                    for kk in range(KC // 2):
                        nc.tensor.matmul(
                            pm_f[:],
                            w2_sb[:, 2 * kk : 2 * kk + 2, es],
                            u_t[:, 2 * kk : 2 * kk + 2, :],
                            start=(kk == 0), stop=(kk == KC // 2 - 1),
                            perf_mode=DR,
                        )
                    outf = outp.tile([128, TT], bf16)
                    nc.vector.scalar_tensor_tensor(
                        outf[:], pm_f[:], 1.0 / WS, h_cur[:, e, :], OP.mult, OP.add
                    )
                    nc.sync.dma_start(out_sT[es, ts], outf[:])
                    if e == 0 and i + 1 < NT:
                        hn_t = norm_b(h_map[i + 1], rms_next)

    nc.compile()
    return nc


_CACHE = {}
last_perf = {}


def _get_programs():
    if "l1" not in _CACHE:
        _CACHE["l1"] = build_l1()
        _CACHE["l2"] = build_l2()
    return _CACHE["l1"], _CACHE["l2"]


def kernel(inp, Wg, Wc, w1, w2, w3, ln_w, ffn_w):
    import os
    import time

    trace = bool(int(os.environ.get("MINGRU_TRACE", "0")))
    nc1, nc2 = _get_programs()

    inp = np.asarray(inp, np.float32)
    ln_w = np.asarray(ln_w, np.float32)
    ffn_w = np.asarray(ffn_w, np.float32)
    # fold norm scales into the matmul weights (exact); fp8 weights carry a
    # x32 scale that the kernel divides back out post-matmul
    def pack_gate(w):
        # W^T [d, e] -> [EC, 128p, KC, 128] with d = k*128 + p
        wT = (np.asarray(w, np.float32) * ln_w).T.astype(bf16_np)
        return np.ascontiguousarray(
            wT.reshape(KC, 128, EC, 128).transpose(2, 1, 0, 3)
        )

    wgP = pack_gate(Wg)
    wcP = pack_gate(Wc)
    w1T = np.ascontiguousarray((np.asarray(w1, np.float32) * ffn_w).T * WS).astype(fp8_np)
    w3T = np.ascontiguousarray((np.asarray(w3, np.float32) * ffn_w).T * WS).astype(fp8_np)
    w2T = np.ascontiguousarray(np.asarray(w2, np.float32).T * WS).astype(fp8_np)

    # per-token 1/rms of the input, replicated across partitions (exact f32
    # preprocessing, like the weight folds)
    inv_r = 1.0 / np.sqrt((inp * inp).mean(axis=-1) + EPS)   # [B, L]

    in1 = []
    for c in range(NCORES):
        b, half = divmod(c, 2)
        sl = slice(half * T, (half + 1) * T)
        in1.append(
            {
                "xT": np.ascontiguousarray(inp[b, sl, :].T),
                "inv": np.ascontiguousarray(
                    np.broadcast_to(inv_r[b, sl][None, :], (128, T))
                ),
                "wgP": wgP,
                "wcP": wcP,
            }
        )
    t0 = time.time()
    r1 = run_bass_kernel_spmd(nc1, in1, core_ids=list(range(NCORES)), trace=trace)
    t1 = time.time()

    zeros = np.zeros((128, EC), np.float32)
    in2 = []
    for c in range(NCORES):
        b, half = divmod(c, 2)
        carry = r1.results[2 * b]["hlast"] if half == 1 else zeros
        in2.append(
            {
                "ax_in": r1.results[c]["ax_T"],
                "carry": np.ascontiguousarray(carry),
                "w1T": w1T,
                "w3T": w3T,
                "w2T": w2T,
            }
        )
    t2 = time.time()
    r2 = run_bass_kernel_spmd(nc2, in2, core_ids=list(range(NCORES)), trace=trace)
    t3 = time.time()

    out = np.empty((B, L, D), np.float32)
    hx = np.empty((B, L, D), np.float32)
    for c in range(NCORES):
        b, half = divmod(c, 2)
        sl = slice(half * T, (half + 1) * T)
        out[b, sl, :] = r2.results[c]["out_sT"].T.astype(np.float32)
        hx[b, sl, :] = r2.results[c]["hx_sT"].T.astype(np.float32)

    last_perf["r1"] = r1
    last_perf["r2"] = r2
    last_perf["t_l1"] = t1 - t0
    last_perf["t_l2"] = t3 - t2
    return out, hx


# revision 29
# speedup vs baseline: 1.1126x; 1.1126x over previous
# MinGRU block kernel for 8 Trainium2 NeuronCores (Bass/Tile).
#
# Reference computation (B=4, L=8192, D=1024, f32):
#   norm = rmsnorm(inp, ln_w)
#   beta = sigmoid(norm @ Wg.T); hx_hat = norm @ Wc.T
#   a = 1-beta; x = beta*hx_hat
#   h = assoc_scan(h_t = a_t*h_{t-1} + x_t) along L
#   out = h + SwiGLU_FFN(rmsnorm(h, ffn_w));  returns (out, h)
#
# Sharding: 8 cores = 4 batches x 2 sequence halves. Scan carry between the
# halves is exchanged on host between two launches.
#   L1: xn = inp^T * inv_rms (host-precomputed scale) -> gate matmuls (bf16)
#       -> a = sigmoid(-g) on scalar engine, x = beta*c on DVE -> local scan
#       (for the carry) -> packed a,x bf16 to DRAM.
#   L2: scan(a, x, init=carry) -> rmsnorm (PE ones-matmul + gpsimd bcast)
#       -> SwiGLU FFN in fp8e4 DoubleRow perf mode (2x PE) -> out^T bf16.
# Both launches software-pipeline tile i+1's load/scan/norm phases under
# tile i's matmul phases so the tensor engine never waits on the scan chain.
# ln_w / ffn_w / fp8 scale (x32) are folded into the weights on host.

import sys

sys.path.insert(0, "/opt/trn_rl_repo")

import numpy as np
import ml_dtypes

import concourse.bass as bass
import concourse.tile as tile
from concourse import mybir, bacc
from concourse.bass_utils import run_bass_kernel_spmd

B, L, D = 4, 8192, 1024
NCORES = 8
T = L // 2        # tokens per core
TT = 512          # token tile
NT = T // TT      # 8 token tiles per core
KC = D // 128     # contraction chunks
EC = D // 128     # output-channel chunks
EPS = 1e-6
WS = 32.0         # fp8 weight scale (power of 2)

f32 = mybir.dt.float32
bf16 = mybir.dt.bfloat16
fp8 = mybir.dt.float8e4
AF = mybir.ActivationFunctionType
OP = mybir.AluOpType
DR = mybir.MatmulPerfMode.DoubleRow
bf16_np = ml_dtypes.bfloat16
fp8_np = ml_dtypes.float8_e4m3


def build_l1():
    nc = bacc.Bacc(None, target_bir_lowering=False)
    xT_d = nc.dram_tensor("xT", [D, T], f32, kind="ExternalInput")
    inv_d = nc.dram_tensor("inv", [128, T], f32, kind="ExternalInput")
    # weights pre-packed on host as [EC, 128, KC, 128] (e-chunk major) so the
    # e=0 chains only wait for 0.5 MiB of weight DMA at launch
    wgP_d = nc.dram_tensor("wgP", [EC, 128, KC, 128], bf16, kind="ExternalInput")
    wcP_d = nc.dram_tensor("wcP", [EC, 128, KC, 128], bf16, kind="ExternalInput")
    ax_T = nc.dram_tensor("ax_T", [D, 2, T], bf16, kind="ExternalOutput")
    hlast = nc.dram_tensor("hlast", [128, EC], f32, kind="ExternalOutput")

    xT_r = xT_d[:].rearrange("(k p) t -> p k t", p=128)

    with tile.TileContext(nc) as tc:
        with (
            tc.tile_pool(name="wpool", bufs=1) as wpool,
            tc.tile_pool(name="xin", bufs=2) as xin,
            tc.tile_pool(name="invp", bufs=2) as invp,
            tc.tile_pool(name="xnp", bufs=2) as xnp,
            tc.tile_pool(name="gate", bufs=4) as gate,
            tc.tile_pool(name="scan", bufs=2) as scanp,
            tc.tile_pool(name="per", bufs=1) as per,
            tc.tile_pool(name="psg", bufs=2, space=bass.MemorySpace.PSUM) as psg,
            tc.tile_pool(name="psc", bufs=2, space=bass.MemorySpace.PSUM) as psc,
        ):
            hprev = per.tile([128, EC], f32)
            nc.vector.memset(hprev[:], 0.0)

            def load_tile(i):
                # per-k DMA chunks so the first chains can start before the
                # whole tile has landed
                ts = slice(i * TT, (i + 1) * TT)
                invt = invp.tile([128, TT], f32)
                nc.sync.dma_start(invt[:], inv_d[:, ts])
                xt = xin.tile([128, KC, TT], f32)
                for k in range(KC):
                    nc.sync.dma_start(xt[:, k, :], xT_r[:, k, ts])
                return xt, invt

            def make_xn(xt, invt, eng):
                # steady state runs on gpsimd: not latency-critical (issued a
                # tile ahead) and keeps the DVE free for the scan chain
                xn = xnp.tile([128, KC, TT], bf16)
                for k in range(KC):
                    eng.tensor_mul(xn[:, k, :], xt[:, k, :], invt[:])
                return xn

            # DMA order: inv+first x chunk, then the e=0 weights (so the first
            # chain starts ~2us in), then the rest of the inputs and weights
            inv0 = invp.tile([128, TT], f32)
            nc.sync.dma_start(inv0[:], inv_d[:, 0:TT])
            xt0 = xin.tile([128, KC, TT], f32)
            nc.sync.dma_start(xt0[:, 0, :], xT_r[:, 0, 0:TT])
            wg_sb = wpool.tile([128, EC, KC, 128], bf16)
            wc_sb = wpool.tile([128, EC, KC, 128], bf16)
            nc.sync.dma_start(wg_sb[:, 0], wgP_d[:][0])
            nc.sync.dma_start(wc_sb[:, 0], wcP_d[:][0])
            for k in range(1, KC):
                nc.sync.dma_start(xt0[:, k, :], xT_r[:, k, 0:TT])
            for e in range(1, EC):
                nc.sync.dma_start(wg_sb[:, e], wgP_d[:][e])
                nc.sync.dma_start(wc_sb[:, e], wcP_d[:][e])
            nxt = (xt0, inv0)
            xn = make_xn(*nxt, nc.vector)

            for i in range(NT):
                ts = slice(i * TT, (i + 1) * TT)
                if i + 1 < NT:
                    nxt = load_tile(i + 1)
                xn_cur, xn = xn, None
                for e in range(EC):
                    es = slice(e * 128, (e + 1) * 128)
                    pm_g = psg.tile([128, TT], f32)
                    for k in range(KC):
                        nc.tensor.matmul(
                            pm_g[:], wg_sb[:, e, k, :], xn_cur[:, k, :],
                            start=(k == 0), stop=(k == KC - 1),
                        )
                    pm_c = psc.tile([128, TT], f32)
                    for k in range(KC):
                        nc.tensor.matmul(
                            pm_c[:], wc_sb[:, e, k, :], xn_cur[:, k, :],
                            start=(k == 0), stop=(k == KC - 1),
                        )
                    ax = gate.tile([128, 2, TT], bf16, tag="ax")
                    nc.scalar.activation(ax[:, 0, :], pm_g[:], AF.Sigmoid, scale=-1.0)
                    beta = gate.tile([128, TT], bf16, tag="beta")
                    nc.scalar.activation(beta[:], pm_g[:], AF.Sigmoid)
                    nc.vector.tensor_mul(ax[:, 1, :], beta[:], pm_c[:])
                    nc.sync.dma_start(ax_T[es, :, ts], ax[:])

                    h = scanp.tile([128, TT], f32)
                    nc.vector.tensor_tensor_scan(
                        h[:], ax[:, 0, :], ax[:, 1, :], hprev[:, e : e + 1],
                        OP.mult, OP.add,
                    )
                    nc.vector.tensor_copy(hprev[:, e : e + 1], h[:, TT - 1 : TT])
                    if e == 0 and i + 1 < NT:
                        xn = make_xn(*nxt, nc.gpsimd)

            nc.sync.dma_start(hlast[:], hprev[:])

    nc.compile()
    return nc


def build_l2():
    nc = bacc.Bacc(None, target_bir_lowering=False)
    ax_in = nc.dram_tensor("ax_in", [D, 2, T], bf16, kind="ExternalInput")
    carry_d = nc.dram_tensor("carry", [128, EC], f32, kind="ExternalInput")
    w1T_d = nc.dram_tensor("w1T", [D, D], fp8, kind="ExternalInput")
    w3T_d = nc.dram_tensor("w3T", [D, D], fp8, kind="ExternalInput")
    w2T_d = nc.dram_tensor("w2T", [D, D], fp8, kind="ExternalInput")
    out_sT = nc.dram_tensor("out_sT", [D, T], bf16, kind="ExternalOutput")
    hx_sT = nc.dram_tensor("hx_sT", [D, T], bf16, kind="ExternalOutput")

    with tile.TileContext(nc) as tc:
        with (
            tc.tile_pool(name="wpool", bufs=1) as wpool,
            tc.tile_pool(name="ax", bufs=4) as axp,
            tc.tile_pool(name="hp", bufs=3) as hp,
            tc.tile_pool(name="hxb", bufs=4) as hxbp,
            tc.tile_pool(name="nrm", bufs=2) as nrm,
            tc.tile_pool(name="hnp", bufs=2) as hnp,
            tc.tile_pool(name="ffn", bufs=2) as ffnp,
            tc.tile_pool(name="outp", bufs=4) as outp,
            tc.tile_pool(name="per", bufs=1) as per,
            tc.tile_pool(name="ps1", bufs=2, space=bass.MemorySpace.PSUM) as ps1,
            tc.tile_pool(name="ps3", bufs=2, space=bass.MemorySpace.PSUM) as ps3,
            tc.tile_pool(name="psf", bufs=2, space=bass.MemorySpace.PSUM) as psf,
            tc.tile_pool(name="pss", bufs=1, space=bass.MemorySpace.PSUM) as pss,
            tc.tile_pool(name="psr", bufs=1, space=bass.MemorySpace.PSUM) as psr,
        ):
            hprev = per.tile([128, EC], f32)
            carry_sb = per.tile([128, EC], f32)
            nc.sync.dma_start(carry_sb[:], carry_d[:])
            eps_row = per.tile([1, 1], f32)
            nc.vector.memset(eps_row[:], EPS)
            ones_b = per.tile([128, 1], bf16)
            nc.vector.memset(ones_b[:], 1.0)
            ones_c = per.tile([1, 128], f32)
            nc.vector.memset(ones_c[:], 1.0)

            def scan_chunk(i, c, h_t):
                # DMA load + DVE scan + hx cast/write (gpsimd) + square
                # (scalar) for one channel chunk of tile i
                ts = slice(i * TT, (i + 1) * TT)
                cs = slice(c * 128, (c + 1) * 128)
                ax = axp.tile([128, 2, TT], bf16)
                nc.sync.dma_start(ax[:], ax_in[cs, :, ts])
                init = carry_sb[:, c : c + 1] if i == 0 else hprev[:, c : c + 1]
                nc.vector.tensor_tensor_scan(
                    h_t[:, c, :], ax[:, 0, :], ax[:, 1, :], init, OP.mult, OP.add
                )
                nc.vector.tensor_copy(hprev[:, c : c + 1], h_t[:, c, TT - 1 : TT])
                hsq = nrm.tile([128, TT], bf16, tag=f"hsq{c}")
                nc.scalar.square(hsq[:], h_t[:, c, :])
                return hsq

            def hx_writeback(i, h_t):
                # bf16 cast on scalar (batched off the per-chunk critical
                # path) + DMA out
                ts = slice(i * TT, (i + 1) * TT)
                for c in range(EC):
                    cs = slice(c * 128, (c + 1) * 128)
                    hxb = hxbp.tile([128, TT], bf16)
                    nc.scalar.activation(hxb[:], h_t[:, c, :], AF.Copy)
                    nc.sync.dma_start(hx_sT[cs, ts], hxb[:])

            def ssq_sqrt(hsqs):
                # PE ones-matmul ssq reduction + scalar sqrt -> rms row
                pm_ssq = pss.tile([1, TT], f32)
                for c in range(EC):
                    nc.tensor.matmul(
                        pm_ssq[:], ones_b[:], hsqs[c][:],
                        start=(c == 0), stop=(c == EC - 1),
                    )
                rms_row = nrm.tile([1, TT], f32, tag="rmsrow")
                nc.scalar.activation(
                    rms_row[:], pm_ssq[:], AF.Sqrt, scale=1.0 / D, bias=eps_row[:]
                )
                return rms_row

            def norm_b(h_t, rms_row):
                # PE broadcast -> 1/rms (fast approx) -> hn in fp8 (DVE)
                pm_rms = psr.tile([128, TT], f32)
                nc.tensor.matmul(pm_rms[:], ones_c[:], rms_row[:], start=True, stop=True)
                inv_bc = nrm.tile([128, TT], f32, tag="invbc")
                nc.vector.reciprocal_approx_fast(inv_bc[:], pm_rms[:])
                hn_t = hnp.tile([128, KC, TT], fp8)
                for c in range(EC):
                    nc.vector.tensor_mul(hn_t[:, c, :], h_t[:, c, :], inv_bc[:])
                return hn_t

            # prologue: tile 0 scan + full norm, tile 1 scan; weights DMA
            # after tile 0's a,x loads (they are not needed until the FFN)
            h_map, hsq_map = {}, {}
            h_map[0] = hp.tile([128, EC, TT], f32, name="h_t", tag="h_t")
            hsq_map[0] = [scan_chunk(0, c, h_map[0]) for c in range(EC)]
            w1_sb = wpool.tile([128, KC, D], fp8)
            nc.sync.dma_start(w1_sb[:], w1T_d[:].rearrange("(k p) e -> p k e", p=128))
            w3_sb = wpool.tile([128, KC, D], fp8)
            nc.sync.dma_start(w3_sb[:], w3T_d[:].rearrange("(k p) e -> p k e", p=128))
            w2_sb = wpool.tile([128, KC, D], fp8)
            nc.sync.dma_start(w2_sb[:], w2T_d[:].rearrange("(k p) e -> p k e", p=128))
            hn_t = norm_b(h_map[0], ssq_sqrt(hsq_map[0]))
            hx_writeback(0, h_map[0])
            if NT > 1:
                h_map[1] = hp.tile([128, EC, TT], f32, name="h_t", tag="h_t")
                hsq_map[1] = [scan_chunk(1, c, h_map[1]) for c in range(EC)]

            for i in range(NT):
                ts = slice(i * TT, (i + 1) * TT)
                h_cur, hn_cur = h_map.pop(i), hn_t

                if i + 2 < NT:
                    h_map[i + 2] = hp.tile([128, EC, TT], f32, name="h_t", tag="h_t")
                    hsq_map[i + 2] = []

                # FFN stage 1: u = silu(w1@hn/32) * (w3@hn/32), fp8 DoubleRow.
                # Tile i+2's scan chunks interleave (two-tile skew) so the
                # scan chain never races the FFN that consumes it.
                u_t = ffnp.tile([128, KC, TT], fp8, tag="u")
                for e in range(EC):
                    es = slice(e * 128, (e + 1) * 128)
                    pm_1 = ps1.tile([128, TT], f32)
                    for kk in range(KC // 2):
                        nc.tensor.matmul(
                            pm_1[:],
                            w1_sb[:, 2 * kk : 2 * kk + 2, es],
                            hn_cur[:, 2 * kk : 2 * kk + 2, :],
                            start=(kk == 0), stop=(kk == KC // 2 - 1),
                            perf_mode=DR,
                        )
                    sil = ffnp.tile([128, TT], bf16, tag="sil")
                    nc.scalar.activation(sil[:], pm_1[:], AF.Silu, scale=1.0 / WS)
                    pm_3 = ps3.tile([128, TT], f32)
                    for kk in range(KC // 2):
                        nc.tensor.matmul(
                            pm_3[:],
                            w3_sb[:, 2 * kk : 2 * kk + 2, es],
                            hn_cur[:, 2 * kk : 2 * kk + 2, :],
                            start=(kk == 0), stop=(kk == KC // 2 - 1),
                            perf_mode=DR,
                        )
                    nc.vector.scalar_tensor_tensor(
                        u_t[:, e, :], pm_3[:], 1.0 / WS, sil[:], OP.mult, OP.mult
                    )
                    if i + 2 < NT:
                        hsq_map[i + 2].append(scan_chunk(i + 2, e, h_map[i + 2]))

                # rms row for tile i+1 (PE + scalar; inputs were ready a full
                # tile ago), then its hx writeback under the w2 window
                if i + 1 < NT:
                    rms_next = ssq_sqrt(hsq_map.pop(i + 1))
                    hx_writeback(i + 1, h_map[i + 1])

                # FFN stage 2: ff = w2@u/32 + h, store out^T (bf16); tile
                # i+1's broadcast/recip/hn run under the remaining w2 chains
                for e in range(EC):
                    es = slice(e * 128, (e + 1) * 128)
                    pm_f = psf.tile([128, TT], f32)
                    for kk in range(KC // 2):
                        nc.tensor.matmul(
                            pm_f[:],
                            w2_sb[:, 2 * kk : 2 * kk + 2, es],
                            u_t[:, 2 * kk : 2 * kk + 2, :],
                            start=(kk == 0), stop=(kk == KC // 2 - 1),
                            perf_mode=DR,
                        )
                    outf = outp.tile([128, TT], bf16)
                    nc.vector.scalar_tensor_tensor(
                        outf[:], pm_f[:], 1.0 / WS, h_cur[:, e, :], OP.mult, OP.add
                    )
                    nc.sync.dma_start(out_sT[es, ts], outf[:])
                    if e == 0 and i + 1 < NT:
                        hn_t = norm_b(h_map[i + 1], rms_next)

    nc.compile()
    return nc


_CACHE = {}
last_perf = {}


def _get_programs():
    if "l1" not in _CACHE:
        _CACHE["l1"] = build_l1()
        _CACHE["l2"] = build_l2()
    return _CACHE["l1"], _CACHE["l2"]


def kernel(inp, Wg, Wc, w1, w2, w3, ln_w, ffn_w):
    import os
    import time

    trace = bool(int(os.environ.get("MINGRU_TRACE", "0")))
    nc1, nc2 = _get_programs()

    inp = np.asarray(inp, np.float32)
    ln_w = np.asarray(ln_w, np.float32)
    ffn_w = np.asarray(ffn_w, np.float32)
    # fold norm scales into the matmul weights (exact); fp8 weights carry a
    # x32 scale that the kernel divides back out post-matmul
    def pack_gate(w):
        # W^T [d, e] -> [EC, 128p, KC, 128] with d = k*128 + p
        wT = (np.asarray(w, np.float32) * ln_w).T.astype(bf16_np)
        return np.ascontiguousarray(
            wT.reshape(KC, 128, EC, 128).transpose(2, 1, 0, 3)
        )

    wgP = pack_gate(Wg)
    wcP = pack_gate(Wc)
    w1T = np.ascontiguousarray((np.asarray(w1, np.float32) * ffn_w).T * WS).astype(fp8_np)
    w3T = np.ascontiguousarray((np.asarray(w3, np.float32) * ffn_w).T * WS).astype(fp8_np)
    w2T = np.ascontiguousarray(np.asarray(w2, np.float32).T * WS).astype(fp8_np)

    # per-token 1/rms of the input, replicated across partitions (exact f32
    # preprocessing, like the weight folds)
    inv_r = 1.0 / np.sqrt((inp * inp).mean(axis=-1) + EPS)   # [B, L]

    in1 = []
    for c in range(NCORES):
        b, half = divmod(c, 2)
        sl = slice(half * T, (half + 1) * T)
        in1.append(
            {
                "xT": np.ascontiguousarray(inp[b, sl, :].T),
                "inv": np.ascontiguousarray(
                    np.broadcast_to(inv_r[b, sl][None, :], (128, T))
                ),
                "wgP": wgP,
                "wcP": wcP,
            }
        )
    t0 = time.time()
    r1 = run_bass_kernel_spmd(nc1, in1, core_ids=list(range(NCORES)), trace=trace)
    t1 = time.time()

    zeros = np.zeros((128, EC), np.float32)
    in2 = []
    for c in range(NCORES):
        b, half = divmod(c, 2)
        carry = r1.results[2 * b]["hlast"] if half == 1 else zeros
        in2.append(
            {
                "ax_in": r1.results[c]["ax_T"],
                "carry": np.ascontiguousarray(carry),
                "w1T": w1T,
                "w3T": w3T,
                "w2T": w2T,
            }
        )
    t2 = time.time()
    r2 = run_bass_kernel_spmd(nc2, in2, core_ids=list(range(NCORES)), trace=trace)
    t3 = time.time()

    out = np.empty((B, L, D), np.float32)
    hx = np.empty((B, L, D), np.float32)
    for c in range(NCORES):
        b, half = divmod(c, 2)
        sl = slice(half * T, (half + 1) * T)
        out[b, sl, :] = r2.results[c]["out_sT"].T.astype(np.float32)
        hx[b, sl, :] = r2.results[c]["hx_sT"].T.astype(np.float32)

    last_perf["r1"] = r1
    last_perf["r2"] = r2
    last_perf["t_l1"] = t1 - t0
    last_perf["t_l2"] = t3 - t2
    return out, hx
